# revision 13
# baseline (speedup 1.0000x reference)
"""GNN message-passing kernel for 8 Trainium2 NeuronCores.

Strategy (edge/data parallel per the sharding hint):
  - Host sorts edges by destination node and cuts the sorted stream into
    fixed-size "windows": each window spans <= 64 destination nodes and
    exactly G*128 edges (padded with dummy edges).  Windows are assigned to
    cores in contiguous blocks, so each core owns a contiguous dst range and
    no inter-core reduction is needed.
  - Each core gathers node features for its edges (indirect DMA from
    replicated bf16 node tables), runs both edge MLPs in bf16 on TensorE,
    forms the five gate tensors, and segment-sums per-window node
    contributions with onehot matmuls (dir components folded into the
    onehot via fused tensor_scalar ops).
  - The cross-product term uses the identity
        sum_e cp[e,f] * cross(dir_e, eq[dst_e,f,:]) = cross(A[n,f,:], eq[n,f,:])
    with A[n,f,:] = sum_e cp[e,f]*dir_e  -- so eq[dst] is never gathered.
  - Device emits raw dv/ds window rows and (inv_edge + de); host scatter-adds
    window rows into the full-size outputs (data movement only).
"""

import sys

sys.path.insert(0, "/opt/trn_rl_repo")

from contextlib import ExitStack

import numpy as np
import ml_dtypes

import concourse.bass as bass
import concourse.mybir as mybir
import concourse.tile as tile
import concourse.bacc as bacc
from concourse.bass_utils import run_bass_kernel_spmd

BF16 = ml_dtypes.bfloat16
F32 = np.float32

# Problem constants (hardcoded per harness contract)
N_NODES = 10000
N_EDGES = 200000
F = 128
LENGTH = 10.0
P = 128
NCORES = 8

W = 64          # nodes per window
G = 10          # 128-edge chunks per window
CH = 128        # edges per chunk
TPC = 4         # chunks per tile
TILE_E = CH * TPC  # 512 edges per tile

_nc_cache = {}


# --------------------------------------------------------------------------
# host-side graph partitioning
# --------------------------------------------------------------------------

def _cut_windows(dst_sorted):
    """Cut the dst-sorted edge stream into windows of <=W nodes, <=G*CH edges."""
    E = dst_sorted.shape[0]
    wins = []  # (base_node, lo, hi) edge slice [lo, hi)
    i = 0
    max_e = G * CH
    while i < E:
        base = int(dst_sorted[i])
        hi_node = base + W
        j = min(i + max_e, E)
        # shrink j so all dst < hi_node  (dst_sorted ascending)
        j = int(np.searchsorted(dst_sorted[i:j], hi_node, side="left")) + i
        wins.append((base, i, j))
        i = j
    return wins


def _pack_idx(idx_i16):
    """Pack per-tile gather indices: element i -> [i%16, i//16], tiled x8."""
    n = idx_i16.shape[0]
    assert n % 16 == 0
    return np.tile(idx_i16.reshape(-1, 16).T, (8, 1))


def _prepare(inputs):
    src = np.asarray(inputs["edge_index"][0], np.int64)
    dst = np.asarray(inputs["edge_index"][1], np.int64)
    inv_node = np.asarray(inputs["invariant_node_features"], F32)
    eq_node = np.asarray(inputs["equivariant_node_features"], F32)
    inv_edge = np.asarray(inputs["invariant_edge_features"], F32)
    edge_dist = np.asarray(inputs["edge_dist"], F32)
    edge_dir = np.asarray(inputs["edge_dir"], F32)

    E = src.shape[0]
    perm = np.argsort(dst, kind="stable")
    dst_s = dst[perm]

    wins = _cut_windows(dst_s)
    nwin_total = len(wins)
    # pad total window count to a multiple of NCORES (dummy empty windows)
    nwin_pad = (-nwin_total) % NCORES
    nwin_per_core = (nwin_total + nwin_pad) // NCORES
    # NCH must be a multiple of TPC; G*nwin_per_core % 4: make nwin_per_core even
    if (nwin_per_core * G) % TPC != 0:
        add = 1
        while ((nwin_per_core + add) * G) % TPC != 0 or \
                (nwin_total + nwin_pad + add * NCORES) % NCORES != 0:
            add += 1
        nwin_per_core += add
    total_windows = nwin_per_core * NCORES
    NCH = nwin_per_core * G
    EP = NCH * CH  # padded edges per core

    # per-core window lists (contiguous)
    core_wins = []
    k = 0
    for c in range(NCORES):
        lst = []
        for _ in range(nwin_per_core):
            if k < nwin_total:
                lst.append(wins[k])
            else:
                lst.append((0, 0, 0))  # dummy empty window
            k += 1
        core_wins.append(lst)

    ranks = np.arange(F // 2, dtype=F32)
    eq_flat = eq_node.reshape(N_NODES, F * 3)
    eq_pad = np.vstack([eq_flat, np.zeros((W, F * 3), F32)])

    # replicated tables
    inv_tab_bf = inv_node.astype(BF16)
    eq_tab_planar_bf = np.ascontiguousarray(
        eq_node.transpose(0, 2, 1).reshape(N_NODES, 3 * F)
    ).astype(BF16)

    w_ = {
        "w1p": np.asarray(inputs["phi_W1"], F32).reshape(2, F, F).astype(BF16),
        "w2p": np.asarray(inputs["phi_W2"], F32).astype(BF16),
        "w1w": np.asarray(inputs["w_W1"], F32).astype(BF16),
        "w2w": np.asarray(inputs["w_W2"], F32).astype(BF16),
        "b1p": np.asarray(inputs["phi_b1"], F32).reshape(F, 1),
        "b1w": np.asarray(inputs["w_b1"], F32).reshape(F, 1),
        "b2p": np.ascontiguousarray(np.asarray(inputs["phi_b2"], F32).reshape(5, F).T),
        "b2w": np.ascontiguousarray(np.asarray(inputs["w_b2"], F32).reshape(5, F).T),
        "iota64": np.tile(np.arange(W, dtype=F32)[None, :], (P, 1)).astype(BF16),
    }

    in_maps = []
    metas = []
    for c in range(NCORES):
        lst = core_wins[c]
        src_c = np.zeros(EP, np.int64)
        dstrel_c = np.full(EP, 127.0, F32)
        real_pos = np.zeros(EP, bool)
        orig_eid = np.zeros(EP, np.int64)
        pos = 0
        for (base, lo, hi) in lst:
            n = hi - lo
            if n > 0:
                sl = perm[lo:hi]
                src_c[pos:pos + n] = src[sl]
                dstrel_c[pos:pos + n] = (dst_s[lo:hi] - base).astype(F32)
                real_pos[pos:pos + n] = True
                orig_eid[pos:pos + n] = sl
            pos += G * CH
        assert pos == EP

        sel = orig_eid  # gather source rows for padded stream (pad -> edge 0, masked)
        inv_edge_c = np.where(real_pos[:, None], inv_edge[sel], 0.0).astype(F32)
        dist_c = np.where(real_pos, edge_dist[sel], 1.0).astype(F32)
        dir_c = np.where(real_pos[:, None], edge_dir[sel], 0.0).astype(F32)

        arg = dist_c[None, :] * ranks[:, None] * (np.pi / LENGTH)  # [64, EP]
        peT = np.concatenate([np.sin(arg), np.cos(arg)], 0).astype(BF16)  # [128, EP]

        idx_cols = []
        for t in range(EP // TILE_E):
            idx_cols.append(_pack_idx(src_c[t * TILE_E:(t + 1) * TILE_E].astype(np.int16)))
        idx_packed = np.concatenate(idx_cols, axis=1)  # [128, NCH*8]

        eq_win = np.zeros((nwin_per_core * W, F * 3), F32)
        for wi, (base, lo, hi) in enumerate(lst):
            eq_win[wi * W:(wi + 1) * W] = eq_pad[base:base + W]

        m = {
            "idx": np.ascontiguousarray(idx_packed),
            "inv_edgeT": np.ascontiguousarray(inv_edge_c.T),
            "peT": np.ascontiguousarray(peT),
            "dstrel": np.ascontiguousarray(
                dstrel_c.reshape(NCH, CH).T),  # [128, NCH]
            "dirs": np.ascontiguousarray(
                dir_c.reshape(NCH, CH, 3).transpose(1, 0, 2).reshape(CH, NCH * 3)),
            "eq_win": eq_win,
            "inv_tab": np.asarray(inv_tab_bf),
            "eq_tab": np.asarray(eq_tab_planar_bf),
        }
        for kk, vv in w_.items():
            m[kk] = np.asarray(vv)
        in_maps.append(m)
        metas.append((lst, real_pos, orig_eid))

    shape_key = (NCH, nwin_per_core)
    return in_maps, metas, shape_key, perm


# --------------------------------------------------------------------------
# device kernel
# --------------------------------------------------------------------------

def _build_nc(NCH, NWIN, skip_gather=False, skip_transpose=False,
              skip_seg=False, psum_cfg=0, skip_mlp=False, tr_engine="sync",
              edge_major=True, single_packet=True, gather_batch=1):
    fp32 = mybir.dt.float32
    bf16 = mybir.dt.bfloat16
    Act = mybir.ActivationFunctionType
    Op = mybir.AluOpType

    EP = NCH * CH
    NT = EP // TILE_E

    nc = bacc.Bacc("TRN2", target_bir_lowering=False, debug=False,
                   num_devices=NCORES)

    d_idx = nc.dram_tensor("idx", [P, NCH * 8], mybir.dt.int16, kind="ExternalInput")
    d_invT = nc.dram_tensor("inv_edgeT", [P, EP], fp32, kind="ExternalInput")
    d_peT = nc.dram_tensor("peT", [P, EP], bf16, kind="ExternalInput")
    d_dstrel = nc.dram_tensor("dstrel", [P, NCH], fp32, kind="ExternalInput")
    d_dirs = nc.dram_tensor("dirs", [P, NCH * 3], fp32, kind="ExternalInput")
    d_eqwin = nc.dram_tensor("eq_win", [NWIN * W, F * 3], fp32, kind="ExternalInput")
    d_invtab = nc.dram_tensor("inv_tab", [N_NODES, F], bf16, kind="ExternalInput")
    d_eqtab = nc.dram_tensor("eq_tab", [N_NODES, 3 * F], bf16, kind="ExternalInput")
    d_w1p = nc.dram_tensor("w1p", [2, F, F], bf16, kind="ExternalInput")
    d_w2p = nc.dram_tensor("w2p", [F, 5 * F], bf16, kind="ExternalInput")
    d_w1w = nc.dram_tensor("w1w", [F, F], bf16, kind="ExternalInput")
    d_w2w = nc.dram_tensor("w2w", [F, 5 * F], bf16, kind="ExternalInput")
    d_b1p = nc.dram_tensor("b1p", [F, 1], fp32, kind="ExternalInput")
    d_b1w = nc.dram_tensor("b1w", [F, 1], fp32, kind="ExternalInput")
    d_b2p = nc.dram_tensor("b2p", [F, 5], fp32, kind="ExternalInput")
    d_b2w = nc.dram_tensor("b2w", [F, 5], fp32, kind="ExternalInput")
    d_iota = nc.dram_tensor("iota64", [P, W], bf16, kind="ExternalInput")

    d_outE = nc.dram_tensor("outE", [P, EP], fp32, kind="ExternalOutput")
    d_outDV = nc.dram_tensor("outDV", [NWIN * W, F * 3], fp32, kind="ExternalOutput")
    d_outDS = nc.dram_tensor("outDS", [NWIN * W, F], fp32, kind="ExternalOutput")

    with tile.TileContext(nc) as tc:
        with ExitStack() as ctx:
            const = ctx.enter_context(tc.tile_pool(name="const", bufs=1))
            sb = ctx.enter_context(tc.tile_pool(name="sb", bufs=2))
            sb3 = ctx.enter_context(tc.tile_pool(name="sb3", bufs=3))
            psA = ctx.enter_context(tc.tile_pool(name="psA", bufs=1, space="PSUM"))
            psB = ctx.enter_context(tc.tile_pool(name="psB", bufs=2 if psum_cfg == 1 else 1, space="PSUM"))
            psS = ctx.enter_context(tc.tile_pool(name="psS", bufs=1 if psum_cfg == 1 else 2, space="PSUM"))

            # ---- constants
            c_idx = const.tile([P, NCH * 8], mybir.dt.int16)
            nc.sync.dma_start(out=c_idx[:], in_=d_idx[:, :])
            c_dstrel = const.tile([P, NCH], fp32)
            nc.sync.dma_start(out=c_dstrel[:], in_=d_dstrel[:, :])
            c_dirs = const.tile([P, NCH * 3], fp32)
            nc.sync.dma_start(out=c_dirs[:], in_=d_dirs[:, :])
            c_w1p = const.tile([P, 2, F], bf16)
            nc.sync.dma_start(out=c_w1p[:], in_=d_w1p[:, :, :].rearrange("a b c -> b a c"))
            c_w2p = const.tile([P, 5 * F], bf16)
            nc.sync.dma_start(out=c_w2p[:], in_=d_w2p[:, :])
            c_w1w = const.tile([P, F], bf16)
            nc.sync.dma_start(out=c_w1w[:], in_=d_w1w[:, :])
            c_w2w = const.tile([P, 5 * F], bf16)
            nc.sync.dma_start(out=c_w2w[:], in_=d_w2w[:, :])
            c_b1p = const.tile([P, 1], fp32)
            nc.sync.dma_start(out=c_b1p[:], in_=d_b1p[:, :])
            c_b1w = const.tile([P, 1], fp32)
            nc.sync.dma_start(out=c_b1w[:], in_=d_b1w[:, :])
            c_b2p = const.tile([P, 5], fp32)
            nc.sync.dma_start(out=c_b2p[:], in_=d_b2p[:, :])
            c_b2w = const.tile([P, 5], fp32)
            nc.sync.dma_start(out=c_b2w[:], in_=d_b2w[:, :])
            c_iota = const.tile([P, W], bf16)
            nc.sync.dma_start(out=c_iota[:], in_=d_iota[:, :])

            tile_ctx = {}
            gather_ctx = {}

            def do_gather(tb):
                # one gather instruction pair covering gather_batch tiles
                GB = gather_batch
                ne = GB * TILE_E
                srcT = sb.tile([P, 1, ne], bf16, tag="srcT")
                eq_em = sb.tile([P, GB * TPC, 3 * F], bf16, tag="eq_em")
                if skip_gather:
                    for bb in range(GB):
                        nc.sync.dma_start(
                            out=srcT[:, 0, bb * TILE_E:(bb + 1) * TILE_E],
                            in_=d_peT[:, (tb * GB + bb) * TILE_E:(tb * GB + bb + 1) * TILE_E])
                    for cc in range(GB * TPC):
                        nc.sync.dma_start(out=eq_em[:, cc, :],
                                          in_=d_eqtab[(cc % 64) * P:((cc % 64) + 1) * P, :])
                else:
                    nc.gpsimd.dma_gather(
                        out_ap=srcT[:], in_ap=d_invtab[:, :],
                        idxs_ap=c_idx[:, tb * GB * 32:(tb + 1) * GB * 32],
                        num_idxs=ne, num_idxs_reg=ne, elem_size=F,
                        transpose=True, single_packet=single_packet)
                    nc.gpsimd.dma_gather(
                        out_ap=eq_em[:], in_ap=d_eqtab[:, :],
                        idxs_ap=c_idx[:, tb * GB * 32:(tb + 1) * GB * 32],
                        num_idxs=ne, num_idxs_reg=ne, elem_size=3 * F,
                        transpose=False, single_packet=single_packet)
                gather_ctx[tb] = (srcT, eq_em)

            def do_tile(t):
                cols = slice(t * TILE_E, (t + 1) * TILE_E)
                tb, off = divmod(t, gather_batch)
                if off == 0:
                    do_gather(tb)
                srcT_full, eq_full = gather_ctx[tb]
                srcT = srcT_full[:, :, off * TILE_E:(off + 1) * TILE_E]
                eq_em = eq_full[:, off * TPC:(off + 1) * TPC, :]
                xeF = sb.tile([P, TILE_E], fp32, tag="xeF")
                nc.sync.dma_start(out=xeF[:], in_=d_invT[:, cols])
                peB = sb.tile([P, TILE_E], bf16, tag="peB")
                nc.sync.dma_start(out=peB[:], in_=d_peT[:, cols])
                xeB = sb.tile([P, TILE_E], bf16, tag="xeB")
                nc.vector.tensor_copy(out=xeB[:], in_=xeF[:])
                if skip_mlp:
                    vals = sb.tile([P, TPC, 4 * F], bf16, tag="vals")
                    nc.sync.dma_start(out=vals[:, 0, :], in_=d_peT[:, t * TILE_E:(t + 1) * TILE_E].bitcast(bf16))
                    tile_ctx[t] = (vals, eq_em)
                    return

                h1p_ps = psA.tile([P, TILE_E], fp32, tag="h1p")
                nc.tensor.matmul(h1p_ps[:], c_w1p[:, 0, :], srcT[:, 0, :],
                                 start=True, stop=False)
                nc.tensor.matmul(h1p_ps[:], c_w1p[:, 1, :], xeB[:],
                                 start=False, stop=True)
                h1pT = sb.tile([P, TILE_E], bf16, tag="h1pT")
                nc.scalar.activation(h1pT[:], h1p_ps[:], Act.Silu, bias=c_b1p[:])
                h1w_ps = psA.tile([P, TILE_E], fp32, tag="h1w")
                nc.tensor.matmul(h1w_ps[:], c_w1w[:], peB[:], start=True, stop=True)
                h1wT = sb.tile([P, TILE_E], bf16, tag="h1wT")
                nc.scalar.activation(h1wT[:], h1w_ps[:], Act.Silu, bias=c_b1w[:])

                vals = sb.tile([P, TPC, 4 * F], bf16, tag="vals")
                m_tiles = []
                if edge_major:
                    # [gates|cp|scale|ds] per chunk, edge-major, no transposes.
                    # Requires zero L2 biases (checked by caller).
                    for c in range(TPC):
                        ppe = psB.tile([P, TILE_E], fp32, tag="pp")
                        nc.tensor.matmul(ppe[:], h1pT[:, c * F:(c + 1) * F],
                                         c_w2p[:, 0:4 * F], start=True, stop=True)
                        pwe = psB.tile([P, TILE_E], fp32, tag="pw")
                        nc.tensor.matmul(pwe[:], h1wT[:, c * F:(c + 1) * F],
                                         c_w2w[:, 0:4 * F], start=True, stop=True)
                        twe = sb.tile([P, TILE_E], bf16, tag="tw")
                        nc.scalar.activation(twe[:], pwe[:], Act.Identity)
                        nc.vector.tensor_tensor(out=vals[:, c, :], in0=ppe[:],
                                                in1=twe[:], op=Op.mult)
                    mlp_iter = [4]
                else:
                    mlp_iter = list(range(5))
                for i in mlp_iter:
                    pp = psB.tile([P, TILE_E], fp32, tag="pp")
                    nc.tensor.matmul(pp[:], c_w2p[:, i * F:(i + 1) * F], h1pT[:],
                                     start=True, stop=True)
                    pw = psB.tile([P, TILE_E], fp32, tag="pw")
                    nc.tensor.matmul(pw[:], c_w2w[:, i * F:(i + 1) * F], h1wT[:],
                                     start=True, stop=True)
                    tw = sb.tile([P, TILE_E], bf16, tag="tw")
                    nc.scalar.activation(tw[:], pw[:], Act.Identity,
                                         bias=c_b2w[:, i:i + 1])
                    if i < 4:
                        mi = sb.tile([P, TILE_E], bf16, tag=f"m{i}")
                        nc.vector.scalar_tensor_tensor(
                            out=mi[:], in0=pp[:], scalar=c_b2p[:, i:i + 1],
                            in1=tw[:], op0=Op.add, op1=Op.mult)
                        m_tiles.append(mi)
                    else:
                        deF = sb.tile([P, TILE_E], fp32, tag="deF")
                        nc.vector.scalar_tensor_tensor(
                            out=deF[:], in0=pp[:], scalar=c_b2p[:, i:i + 1],
                            in1=tw[:], op0=Op.add, op1=Op.mult)
                        outE_t = sb.tile([P, TILE_E], fp32, tag="outE")
                        nc.vector.tensor_tensor(out=outE_t[:], in0=deF[:],
                                                in1=xeF[:], op=Op.add)
                        nc.sync.dma_start(out=d_outE[:, cols], in_=outE_t[:])

                # fallback vals per chunk: [ds | scale | cp | gates]
                order = [3, 2, 1, 0]  # m index per val slot
                if edge_major:
                    pass
                elif skip_transpose:
                    for qi, mq in enumerate(order):
                        nc.sync.dma_start(out=vals[:, qi, :],
                                          in_=m_tiles[mq][:].rearrange("p a -> p a"))
                elif tr_engine == "act":
                    for c in range(TPC):
                        for qi, mq in enumerate(order):
                            nc.scalar.dma_start_transpose(
                                vals[:, c, qi * F:(qi + 1) * F],
                                m_tiles[mq][:, c * F:(c + 1) * F])
                else:
                    for c in range(TPC):
                        for qi, mq in enumerate(order):
                            nc.sync.dma_start_transpose(
                                vals[:, c, qi * F:(qi + 1) * F],
                                m_tiles[mq][:, c * F:(c + 1) * F])
                tile_ctx[t] = (vals, eq_em)

            for w in range(NWIN):
                seg = psS.tile([W, 7 * F], fp32, tag="seg")
                # layout: [ds | BG0 A0 BG1 A1 BG2 A2]
                for pos in range(G):
                    chg = w * G + pos
                    t, c = divmod(chg, TPC)
                    if c == 0:
                        do_tile(t)
                    vals, eq_em = tile_ctx[t]

                    if skip_seg:
                        continue
                    oh = sb3.tile([P, W], bf16, tag="oh")
                    nc.vector.tensor_scalar(
                        out=oh[:], in0=c_iota[:],
                        scalar1=c_dstrel[:, chg:chg + 1], scalar2=None,
                        op0=Op.is_equal)
                    ohd = sb3.tile([P, 3, W], bf16, tag="ohd")
                    for c3 in range(3):
                        nc.vector.tensor_scalar(
                            out=ohd[:, c3, :], in0=c_iota[:],
                            scalar1=c_dstrel[:, chg:chg + 1],
                            scalar2=c_dirs[:, chg * 3 + c3:chg * 3 + c3 + 1],
                            op0=Op.is_equal, op1=Op.mult)
                    if edge_major:
                        sl_gate = slice(0, F)
                        sl_cp = slice(F, 2 * F)
                        sl_scale = slice(2 * F, 3 * F)
                        sl_ds = slice(3 * F, 4 * F)
                    else:
                        sl_ds = slice(0, F)
                        sl_scale = slice(F, 2 * F)
                        sl_cp = slice(2 * F, 3 * F)
                        sl_gate = slice(3 * F, 4 * F)
                    gg = sb3.tile([P, 3 * F], bf16, tag="gg")
                    for c3 in range(3):
                        nc.vector.tensor_tensor(
                            out=gg[:, c3 * F:(c3 + 1) * F],
                            in0=eq_em[:, c, c3 * F:(c3 + 1) * F],
                            in1=vals[:, c, sl_gate], op=Op.mult)

                    # start=True clears has_written for the WHOLE PSUM bank,
                    # so issue exactly one start=True per bank per window:
                    # ds (col 0, bank 0) and A1 (col 512, bank 1) at pos 0.
                    # All other region writes rely on overwrite-when-clear /
                    # accumulate-when-set semantics.
                    st = pos == 0
                    sp = pos == G - 1
                    # ds: first write of bank 0 in this window
                    nc.tensor.matmul(seg[:, 0:F], oh[:], vals[:, c, sl_ds],
                                     start=st, stop=sp, skip_group_check=True)
                    for c3 in range(3):
                        base = F + c3 * 2 * F
                        a_col = base + F
                        # A1 region starts at col 512 == bank 1 start
                        a_start = st and (a_col == 512)
                        # BG_c += oh @ g_c
                        nc.tensor.matmul(seg[:, base:base + F], oh[:],
                                         gg[:, c3 * F:(c3 + 1) * F],
                                         start=False, stop=False,
                                         skip_group_check=True)
                        # BG_c += ohd_c @ scale ; A_c += ohd_c @ cp
                        nc.tensor.matmul(seg[:, base:base + F], ohd[:, c3, :],
                                         vals[:, c, sl_scale],
                                         start=False, stop=sp,
                                         skip_group_check=True)
                        nc.tensor.matmul(seg[:, a_col:a_col + F],
                                         ohd[:, c3, :], vals[:, c, sl_cp],
                                         start=a_start, stop=sp,
                                         skip_group_check=True)

                # flush + final
                if skip_seg:
                    continue
                acc = sb.tile([W, 7 * F], fp32, tag="acc")
                nc.vector.tensor_copy(out=acc[:], in_=seg[:])
                nc.sync.dma_start(out=d_outDS[w * W:(w + 1) * W, :], in_=acc[:, 0:F])
                eqw = sb.tile([W, 3 * F], fp32, tag="eqw")
                nc.sync.dma_start(out=eqw[:], in_=d_eqwin[w * W:(w + 1) * W, :])
                eq3 = eqw[:].rearrange("p (f c) -> p c f", c=3)
                dv = sb.tile([W, 3 * F], fp32, tag="dv")
                dv3 = dv[:].rearrange("p (f c) -> p c f", c=3)
                for c3 in range(3):
                    a, b = (c3 + 1) % 3, (c3 + 2) % 3
                    A_a = acc[:, F + a * 2 * F + F:F + a * 2 * F + 2 * F]
                    A_b = acc[:, F + b * 2 * F + F:F + b * 2 * F + 2 * F]
                    BG = acc[:, F + c3 * 2 * F:F + c3 * 2 * F + F]
                    t1 = sb3.tile([W, F], fp32, tag="t1")
                    nc.vector.tensor_tensor(out=t1[:], in0=A_a, in1=eq3[:, b, :],
                                            op=Op.mult)
                    s1 = sb3.tile([W, F], fp32, tag="s1")
                    nc.vector.tensor_tensor(out=s1[:], in0=BG, in1=t1[:], op=Op.add)
                    t2 = sb3.tile([W, F], fp32, tag="t2")
                    nc.vector.tensor_tensor(out=t2[:], in0=A_b, in1=eq3[:, a, :],
                                            op=Op.mult)
                    nc.vector.tensor_tensor(out=dv3[:, c3, :], in0=s1[:],
                                            in1=t2[:], op=Op.subtract)
                nc.sync.dma_start(out=d_outDV[w * W:(w + 1) * W, :], in_=dv[:])

    nc.finalize()
    return nc


# --------------------------------------------------------------------------
# entry point
# --------------------------------------------------------------------------

def kernel(**inputs):
    in_maps, metas, shape_key, perm = _prepare(inputs)
    NCH, NWIN = shape_key
    em = bool(np.allclose(np.asarray(inputs["phi_b2"]), 0.0)
              and np.allclose(np.asarray(inputs["w_b2"]), 0.0))
    key = (NCH, NWIN, em)
    if key not in _nc_cache:
        _nc_cache[key] = _build_nc(NCH, NWIN, edge_major=em)
    nc = _nc_cache[key]

    import os
    trace = bool(int(os.environ.get("KERNEL_TRACE", "0")))
    res = run_bass_kernel_spmd(nc, in_maps, core_ids=list(range(NCORES)),
                               trace=trace)
    kernel.last_results = res

    eq_node = np.asarray(inputs["equivariant_node_features"], F32)
    inv_node = np.asarray(inputs["invariant_node_features"], F32)
    inv_edge = np.asarray(inputs["invariant_edge_features"], F32)

    out_eq = eq_node.copy()
    out_in = inv_node.copy()
    out_ed = np.empty_like(inv_edge)

    EP = NCH * CH
    for c in range(NCORES):
        o = res.results[c]
        lst, real_pos, orig_eid = metas[c]
        edge_rows = o["outE"].T  # [EP, F]
        out_ed[orig_eid[real_pos]] = edge_rows[real_pos]
        dv = o["outDV"]
        ds = o["outDS"]
        for wi, (base, lo, hi) in enumerate(lst):
            if hi <= lo:
                continue
            n = min(W, N_NODES - base)
            rows = dv[wi * W: wi * W + n].reshape(n, F, 3)
            out_eq[base:base + n] += rows
            out_in[base:base + n] += ds[wi * W: wi * W + n]

    return out_eq, out_in, out_ed
